# revision 30
# baseline (speedup 1.0000x reference)
"""EnhancedCGConv GNN message-passing kernel for 8 Trainium2 NeuronCores, v3.

Strategy (dst-sharded, zero collectives, host-side layout prep):
  - Edges are bucketed by destination node tile (128 nodes) per core and
    pair-packed: each SBUF column holds TWO edges (rows 0:64 = first edge,
    64:128 = second edge), enabling block-diagonal weights so every matmul
    streams full-rate.
  - The node-feature gather and the recip-scaled one-hot scatter matrices are
    precomputed on the host and streamed as plain sequential DMA (no gpsimd
    gather, no on-device one-hot generation).
  - Per chunk: PE computes h, ew (192ch), ml (192ch); ACT evacuates ew with
    the b2 bias folded into the copy; DVE forms msg = (ew+b2)*ml reading ml
    straight from PSUM; fusion dense (192->64) is applied per 128-edge
    segment BEFORE the one-hot scatter matmul (linearity of segment-mean);
    the mean's 1/cnt is folded into the one-hot values.
  - Epilogue (fusion bias + group-norm + relu) is batched 8 node tiles at a
    time in SBUF.
"""

import os
import sys

import numpy as np

for _p in ("/opt/trn_rl_repo", "/root/.axon_site/_ro/trn_rl_repo"):
    if os.path.isdir(_p) and _p not in sys.path:
        sys.path.insert(0, _p)

from concourse import bacc, bass, mybir, tile  # noqa: E402

P = 128
F = 64          # node feature dim
U = 64          # units
KK = 3          # num kernels
C = KK * U      # 192
G = 16          # group-norm groups
EPS = 1e-5

f32 = mybir.dt.float32
f16 = mybir.dt.float16
i16 = mybir.dt.int16
NPF16 = mybir.dt.np(f16)


class Cfg:
    def __init__(self, N, E, ncores, NSEG):
        assert NSEG % 2 == 0
        self.N = N
        self.E = E
        self.ncores = ncores
        assert N % ncores == 0
        self.NPC = N // ncores
        self.NT = (self.NPC + P - 1) // P
        self.NSEG = NSEG                  # 128-edge segments per tile
        self.ST = NSEG * P                # edge slots per tile
        self.HALF = self.ST // 2          # pair columns per tile
        # chunk sizes over the pair columns (512 wide + remainder)
        cs = [512] * (self.HALF // 512)
        if self.HALF % 512:
            cs.append(self.HALF % 512)
        self.chunks = cs
        self.EPI = 16                     # node tiles per epilogue batch


FULL = Cfg(N=50000, E=800000, ncores=8, NSEG=16)


def _balance_tiles(deg, NT, cap):
    """Partition nodes into NT tiles (last tile short) so every tile's
    degree sum is <= cap. Returns tile assignment or None if infeasible."""
    NPC = deg.size
    caps = np.full(NT, P, np.int64)
    caps[-1] = NPC - (NT - 1) * P
    order = np.argsort(-deg)
    bsum = np.zeros(NT, np.int64)
    bcnt = np.zeros(NT, np.int64)
    assign = np.empty(NPC, np.int32)
    for node in order:
        avail = np.where(bcnt < caps)[0]
        b = avail[np.argmin(bsum[avail])]
        assign[node] = b
        bsum[b] += deg[node]
        bcnt[b] += 1
    nodes_by_bin = [np.where(assign == b)[0].tolist() for b in range(NT)]

    def swap(hi, lo, i, j):
        a, b2 = nodes_by_bin[hi][i], nodes_by_bin[lo][j]
        nodes_by_bin[hi][i], nodes_by_bin[lo][j] = b2, a
        bsum[hi] += deg[b2] - deg[a]
        bsum[lo] += deg[a] - deg[b2]
        assign[a] = lo
        assign[b2] = hi

    rng = np.random.default_rng(12345)
    for _ in range(200000):
        hi = int(np.argmax(bsum))
        if bsum[hi] <= cap:
            return assign
        need = bsum[hi] - cap
        dh = deg[np.asarray(nodes_by_bin[hi])]
        done = False
        for lo in np.argsort(bsum):
            lo = int(lo)
            if lo == hi:
                continue
            room = cap - bsum[lo]
            if room < need:
                break               # ascending order: no later bin has room
            dl = deg[np.asarray(nodes_by_bin[lo])]
            d = dh[:, None] - dl[None, :]
            ok = (d >= need) & (d <= room)
            if ok.any():
                i, j = np.unravel_index(int(np.argmax(ok)), ok.shape)
                swap(hi, lo, i, j)
                done = True
                break
        if not done:
            # partial progress: move load from hi to a random light bin
            los = np.argsort(bsum)[:8]
            lo = int(rng.choice(los))
            if lo == hi:
                return None
            dl = deg[np.asarray(nodes_by_bin[lo])]
            d = dh[:, None] - dl[None, :]
            limit = bsum[hi] - bsum[lo]
            ok = (d > 0) & (d < limit)
            if not ok.any():
                return None
            dv = np.where(ok, d, -1)
            i, j = np.unravel_index(int(np.argmax(dv)), dv.shape)
            swap(hi, lo, i, j)
    return assign if bsum.max() <= cap else None


def layout(cfg):
    """Section offsets (elements) inside blob16 / blob32."""
    NT, HALF, ST = cfg.NT, cfg.HALF, cfg.ST
    lay = {}
    o = 0
    for name, n in (
        ("ef2", NT * P * HALF),      # [NT][128][HALF] pair-packed edge feats
        ("src2", NT * P * HALF),     # [NT][128][HALF] pair-packed src feats
        ("oh", NT * P * ST),         # [NT][128][NSEG*128] recip-scaled onehot
        ("w1bd", P * P),
        ("w2a", P * P),
        ("w2bd", P * P),
        ("wka", P * P),
        ("wkbd", P * P),
        ("fwa", P * U),
        ("fwb", P * U),              # [fwb; fwb]
    ):
        lay[name] = o
        o += n
    lay["len16"] = o
    o = 0
    for name, n in (
        ("b1d", P),        # [b1; b1]
        ("b2a", P),
        ("b2bd", P),       # [b2b; b2b]
        ("fbb", P * U),    # fusion bias broadcast [128, 64]
        ("gamma", U),
        ("beta", U),
    ):
        lay[name] = o
        o += n
    lay["len32"] = o
    return lay


# ---------------------------------------------------------------- host prep

def prepare_inputs(cfg, inputs):
    nf = np.ascontiguousarray(np.asarray(inputs["node_features"], np.float32))
    ei = np.asarray(inputs["edge_indices"])
    src = ei[0].astype(np.int64)
    dst = ei[1].astype(np.int64)
    ef = np.ascontiguousarray(np.asarray(inputs["edge_features"], np.float32))
    W1 = np.asarray(inputs["edge_W1"], np.float32)
    b1 = np.asarray(inputs["edge_b1"], np.float32)
    W2 = np.asarray(inputs["edge_W2"], np.float32)
    b2 = np.asarray(inputs["edge_b2"], np.float32)
    Wk = np.asarray(inputs["W_kernels"], np.float32)
    fW = np.asarray(inputs["fusion_W"], np.float32)
    fb = np.asarray(inputs["fusion_b"], np.float32)
    gamma = np.asarray(inputs["gamma"], np.float32)
    beta = np.asarray(inputs["beta"], np.float32)

    N, E, NPC, NT = cfg.N, cfg.E, cfg.NPC, cfg.NT
    NSEG, ST, HALF = cfg.NSEG, cfg.ST, cfg.HALF
    ncores = cfg.ncores
    lay = layout(cfg)

    core = dst // NPC
    loc = dst - core * NPC

    # degree-balanced node->tile permutation per core (minimizes NSEG)
    deg_all = np.bincount(dst, minlength=N)
    permpos = np.empty(N, np.int64)       # original node -> permuted row
    for c in range(ncores):
        dg = deg_all[c * NPC:(c + 1) * NPC].astype(np.int64)
        assign = _balance_tiles(dg, NT, ST)
        if assign is None:
            raise OverflowError(NSEG + 2)
        pp = np.empty(NPC, np.int64)
        order_n = np.argsort(assign, kind="stable")
        ofs = np.zeros(NT + 1, np.int64)
        np.cumsum(np.bincount(assign, minlength=NT), out=ofs[1:])
        ranks = np.arange(NPC) - ofs[assign[order_n]]
        pp[order_n] = assign[order_n] * P + ranks
        permpos[c * NPC:(c + 1) * NPC] = pp
    global _LAST_PERM
    # original global node -> row in the raw (permuted) device output
    _LAST_PERM = (np.arange(N) // NPC) * NPC + permpos

    ploc = permpos[dst]                   # permuted position within core
    t = ploc >> 7
    n_in_tile = ploc & 127
    key = core * NT + t                      # global tile id
    ntile = ncores * NT
    cnt = np.bincount(key, minlength=ntile)
    if cnt.max() > ST:
        raise OverflowError(NSEG + 2)

    order = np.argsort(key, kind="stable")
    starts = np.zeros(ntile + 1, np.int64)
    np.cumsum(cnt, out=starts[1:])
    rank = np.arange(E, dtype=np.int64) - starts[key[order]]
    ko = key[order]

    # slot arrays [ntile * ST]
    slot = ko * ST + rank
    TOT = ntile * ST
    ef_s = np.zeros((TOT, F), np.float32)
    ef_s[slot] = ef[order]
    src_s = np.zeros((TOT, F), np.float32)
    src_s[slot] = nf[src[order]]

    # recip-scaled one-hot [ntile, 128, NSEG*128] fp16
    cntN = np.bincount(dst, minlength=N).astype(np.float32)
    recip = (1.0 / np.maximum(cntN, 1.0)).astype(NPF16)
    oh = np.zeros(ntile * P * ST, NPF16)
    seg = rank >> 7
    i_in_seg = rank & 127
    flat = (ko * P + i_in_seg) * ST + seg * P + n_in_tile[order]
    oh[flat] = recip[dst[order]]
    oh = oh.view(np.int16).reshape(ntile, P, ST)

    def pack_pairs(xs):
        # [ntile*ST, F] -> [ntile, 128 rows (2xF), HALF cols]
        a = xs.reshape(ntile, NSEG // 2, 2, P, F)
        return np.ascontiguousarray(
            a.transpose(0, 2, 4, 1, 3).astype(NPF16).view(np.int16)
            .reshape(ntile, 2 * F, HALF))

    ef2 = pack_pairs(ef_s)
    src2 = pack_pairs(src_s)

    W_all = np.ascontiguousarray(Wk.transpose(1, 0, 2).reshape(F, C))

    def f16i(x):
        return np.ascontiguousarray(x, np.float32).astype(NPF16).view(np.int16)

    def blockdiag(w):
        z = np.zeros((2 * F, 2 * w.shape[1]), np.float32)
        z[:F, :w.shape[1]] = w
        z[F:, w.shape[1]:] = w
        return z

    w1bd = f16i(blockdiag(W1))
    w2a = f16i(np.concatenate([W2[:, :P], W2[:, :P]], axis=0))
    w2bd = f16i(blockdiag(W2[:, P:]))
    wka = f16i(np.concatenate([W_all[:, :P], W_all[:, :P]], axis=0))
    wkbd = f16i(blockdiag(W_all[:, P:]))
    fwa = f16i(fW[:P])
    fwb = f16i(np.concatenate([fW[P:], fW[P:]], axis=0))

    blob32_tail = np.concatenate([
        np.tile(b1, 2), b2[:P], np.tile(b2[P:], 2),
        np.tile(fb, P), gamma, beta]).astype(np.float32)

    in_maps = []
    for c in range(ncores):
        sl = slice(c * NT, (c + 1) * NT)
        blob16 = np.concatenate([
            ef2[sl].ravel(), src2[sl].ravel(), oh[sl].ravel(),
            w1bd.ravel(), w2a.ravel(), w2bd.ravel(),
            wka.ravel(), wkbd.ravel(), fwa.ravel(), fwb.ravel()])
        assert blob16.size == lay["len16"], (blob16.size, lay["len16"])
        assert blob32_tail.size == lay["len32"]
        in_maps.append({"blob16": blob16, "blob32": blob32_tail})
    return in_maps


# ---------------------------------------------------------------- device IR

def build_nc(cfg, nt_limit=None):
    NPC, NT, NSEG = cfg.NPC, cfg.NT, cfg.NSEG
    ST, HALF, EPI = cfg.ST, cfg.HALF, cfg.EPI
    lay = layout(cfg)
    nc = bacc.Bacc("TRN2", target_bir_lowering=False)

    d16 = nc.dram_tensor("blob16", [lay["len16"]], i16, kind="ExternalInput")
    d32 = nc.dram_tensor("blob32", [lay["len32"]], f32, kind="ExternalInput")
    d_out = nc.dram_tensor("out", [NPC, U], f32, kind="ExternalOutput")

    ACT = mybir.ActivationFunctionType
    ALU = mybir.AluOpType
    AX = mybir.AxisListType

    def v16(name, rows, cols, extra_off=0, rowstride=None):
        return bass.AP(
            tensor=d16, offset=lay[name] + extra_off,
            ap=[[rowstride if rowstride is not None else cols, rows],
                [1, cols]]).bitcast(f16)

    def v32(name, rows, cols, extra_off=0):
        return bass.AP(tensor=d32, offset=lay[name] + extra_off,
                       ap=[[cols, rows], [1, cols]])

    NTS = NT if nt_limit is None else min(NT, nt_limit)

    with tile.TileContext(nc) as tc:
        with tc.tile_pool(name="const", bufs=1) as const, \
             tc.tile_pool(name="efp", bufs=4) as efp, \
             tc.tile_pool(name="srcp", bufs=4) as srcp, \
             tc.tile_pool(name="ohp", bufs=3) as ohp, \
             tc.tile_pool(name="h2p", bufs=3) as h2p, \
             tc.tile_pool(name="ewsp", bufs=4) as ewsp, \
             tc.tile_pool(name="msgp", bufs=3) as msgp, \
             tc.tile_pool(name="fsp", bufs=4) as fsp, \
             tc.tile_pool(name="epi", bufs=2) as epi, \
             tc.tile_pool(name="pscr", bufs=2, space="PSUM") as pscr, \
             tc.tile_pool(name="pew", bufs=2, space="PSUM") as pew, \
             tc.tile_pool(name="pml", bufs=3, space="PSUM") as pml, \
             tc.tile_pool(name="pacc", bufs=1, space="PSUM") as pacc:

            # ---------------- constants ----------------
            def load16(name, rows, cols):
                t_ = const.tile([rows, cols], f16, tag=name)
                nc.sync.dma_start(out=t_[:], in_=v16(name, rows, cols))
                return t_

            w1bd = load16("w1bd", P, P)
            w2a = load16("w2a", P, P)
            w2bd = load16("w2bd", P, P)
            wka = load16("wka", P, P)
            wkbd = load16("wkbd", P, P)
            fwa = load16("fwa", P, U)
            fwb = load16("fwb", P, U)

            def load32(name, rows, cols):
                t_ = const.tile([rows, cols], f32, tag=name)
                nc.sync.dma_start(out=t_[:], in_=v32(name, rows, cols))
                return t_

            b1d = load32("b1d", P, 1)
            b2a = load32("b2a", P, 1)
            b2bd = load32("b2bd", P, 1)
            fbb = load32("fbb", P, U)

            def bcast32(name, w):
                # [P, w*U]: per-channel vector tiled w times along free dim
                t_ = const.tile([P, w * U], f32, tag=name + "b")
                nc.gpsimd.dma_start(
                    out=t_[:],
                    in_=bass.AP(tensor=d32, offset=lay[name],
                                ap=[[0, P], [0, w], [1, U]]))
                return t_

            gammab = bcast32("gamma", EPI)
            betab = bcast32("beta", EPI)
            epsb = const.tile([P, 1], f32)
            nc.vector.memset(epsb[:], EPS)

            # ---------------- epilogue emitter (deferred) ----------------
            pending = []

            def emit_epi(t0, g_n, fu32):
                W = g_n * U
                NG = g_n * G
                D = U // G
                fu3 = fu32[:, :W].rearrange("p (g d) -> p g d", g=NG)
                sumg = epi.tile([P, EPI * G], f32, tag="sumg")
                nc.vector.tensor_reduce(
                    out=sumg[:, :NG], in_=fu3, axis=AX.X, op=ALU.add)
                xsq = epi.tile([P, EPI * U], f32, tag="xsq")
                nc.scalar.activation(out=xsq[:, :W], in_=fu32[:, :W],
                                     func=ACT.Square)
                sqg = epi.tile([P, EPI * G], f32, tag="sqg")
                nc.vector.tensor_reduce(
                    out=sqg[:, :NG],
                    in_=xsq[:, :W].rearrange("p (g d) -> p g d", g=NG),
                    axis=AX.X, op=ALU.add)
                # var = sqg/D - (sumg/D)^2 ; rstd = 1/sqrt(var + eps)
                tmp = epi.tile([P, EPI * G], f32, tag="tmp")
                nc.vector.scalar_tensor_tensor(
                    out=tmp[:, :NG], in0=sumg[:, :NG], scalar=1.0 / (D * D),
                    in1=sumg[:, :NG], op0=ALU.mult, op1=ALU.mult)
                varg = epi.tile([P, EPI * G], f32, tag="varg")
                nc.vector.scalar_tensor_tensor(
                    out=varg[:, :NG], in0=sqg[:, :NG], scalar=1.0 / D,
                    in1=tmp[:, :NG], op0=ALU.mult, op1=ALU.subtract)
                stdg = epi.tile([P, EPI * G], f32, tag="stdg")
                nc.scalar.activation(out=stdg[:, :NG], in_=varg[:, :NG],
                                     func=ACT.Sqrt, bias=epsb[:, 0:1],
                                     scale=1.0)
                rstd = epi.tile([P, EPI * G], f32, tag="rstd")
                nc.vector.reciprocal(out=rstd[:, :NG], in_=stdg[:, :NG])
                mr = epi.tile([P, EPI * G], f32, tag="mr")
                nc.vector.scalar_tensor_tensor(
                    out=mr[:, :NG], in0=sumg[:, :NG], scalar=1.0 / D,
                    in1=rstd[:, :NG], op0=ALU.mult, op1=ALU.mult)
                xn = epi.tile([P, EPI * U], f32, tag="xn")
                xn3 = xn[:, :W].rearrange("p (g d) -> p g d", g=NG)
                rstdb = rstd[:, :NG].unsqueeze(2).broadcast_to([P, NG, D])
                mrb = mr[:, :NG].unsqueeze(2).broadcast_to([P, NG, D])
                nc.vector.tensor_mul(out=xn3, in0=fu3, in1=rstdb)
                nc.vector.tensor_sub(out=xn3, in0=xn3, in1=mrb)
                nc.vector.tensor_mul(out=xn[:, :W], in0=xn[:, :W],
                                     in1=gammab[:, :W])
                nc.vector.tensor_add(out=xn[:, :W], in0=xn[:, :W],
                                     in1=betab[:, :W])
                outf = epi.tile([P, EPI * U], f32, tag="outf")
                nc.scalar.activation(out=outf[:, :W], in_=xn[:, :W],
                                     func=ACT.Relu)
                rows_all = min(g_n * P, NPC - t0 * P)
                full_g = rows_all // P
                if full_g:
                    nc.sync.dma_start(
                        out=bass.AP(tensor=d_out, offset=t0 * P * U,
                                    ap=[[U, P], [P * U, full_g], [1, U]]),
                        in_=outf[:, :full_g * U].rearrange(
                            "p (g u) -> p g u", u=U))
                rem = rows_all - full_g * P
                if rem > 0:
                    nc.sync.dma_start(
                        out=bass.AP(tensor=d_out,
                                    offset=(t0 + full_g) * P * U,
                                    ap=[[U, rem], [1, U]]),
                        in_=outf[:rem, full_g * U:(full_g + 1) * U])

            # ------------- fusion + scatter emitter (deferred) -------------
            fstate = {"acc8": None, "fu32": None}

            def emit_fusion(t, msg, ohc):
                g_i = t % EPI
                g_n = min(EPI, NTS - (t - g_i))
                o_i = t % 8
                if o_i == 0:
                    fstate["acc8"] = pacc.tile([P, 512], f32, space="PSUM",
                                               tag="acc", name="acc8")
                acc8 = fstate["acc8"]
                acc = acc8[:, o_i * U:(o_i + 1) * U]
                for r0 in range(0, NSEG, 8):          # rounds of 8 segments
                    rn = min(8, NSEG - r0)
                    f6 = pscr.tile([P, 512], f32, space="PSUM", tag="scr")
                    for k in range(rn):
                        s = r0 + k
                        jj = s >> 1
                        h = s & 1
                        fo = slice(k * U, (k + 1) * U)
                        nc.tensor.matmul(
                            out=f6[:, fo],
                            lhsT=msg[:, h * HALF + jj * P:
                                     h * HALF + (jj + 1) * P],
                            rhs=fwa[:], start=True, stop=False)
                        nc.tensor.matmul(
                            out=f6[:, fo],
                            lhsT=msg[h * U:(h + 1) * U,
                                     2 * HALF + jj * P:2 * HALF + (jj + 1) * P],
                            rhs=fwb[h * U:(h + 1) * U, :], start=False,
                            stop=True)
                    fsb = fsp.tile([P, 512], f16, tag="fsb")
                    if (r0 == 0) == (t % 2 == 0):
                        nc.scalar.activation(out=fsb[:, :rn * U],
                                             in_=f6[:, :rn * U], func=ACT.Copy)
                    else:
                        nc.vector.tensor_copy(fsb[:, :rn * U], f6[:, :rn * U])
                    for k in range(rn):
                        s = r0 + k
                        nc.tensor.matmul(
                            out=acc, lhsT=ohc[:, s * P:(s + 1) * P],
                            rhs=fsb[:, k * U:(k + 1) * U],
                            start=(s == 0), stop=(s == NSEG - 1))

                # acc evacuation (adds fusion bias)
                if g_i == 0:
                    fstate["fu32"] = epi.tile([P, EPI * U], f32, tag="fu32",
                                              name="fu32")
                fu32 = fstate["fu32"]
                if o_i == 7 or t == NTS - 1:
                    na = o_i + 1
                    nc.vector.tensor_add(
                        out=fu32[:].rearrange("p (g u) -> p g u", u=U)
                            [:, g_i - o_i:g_i + 1, :],
                        in0=acc8[:].rearrange("p (g u) -> p g u", u=U)
                            [:, 0:na, :],
                        in1=fbb[:].unsqueeze(1).broadcast_to([P, na, U]))
                if g_i == g_n - 1:
                    pending.append((t - g_i, g_n, fu32))

            # ---------------- main loop ----------------
            fusq = []
            for t in range(NTS):
                g_i = t % EPI            # index within epilogue batch

                ef2c = efp.tile([P, HALF], f16, tag="ef2")
                nc.sync.dma_start(
                    out=ef2c[:], in_=v16("ef2", P, HALF, extra_off=t * P * HALF))
                srcc = srcp.tile([P, HALF], f16, tag="src2")
                nc.sync.dma_start(
                    out=srcc[:], in_=v16("src2", P, HALF, extra_off=t * P * HALF))
                ohc = ohp.tile([P, ST], f16, tag="oh")
                nc.sync.dma_start(
                    out=ohc[:], in_=v16("oh", P, ST, extra_off=t * P * ST))

                msg = msgp.tile([P, 3 * HALF], f16, tag="msg")
                off = 0
                for ci, sz in enumerate(cfg.chunks):
                    cs = slice(off, off + sz)
                    hp = pscr.tile([P, 512], f32, space="PSUM", tag="scr")
                    nc.tensor.matmul(out=hp[:, :sz], lhsT=w1bd[:],
                                     rhs=ef2c[:, cs], start=True, stop=True)
                    h2 = h2p.tile([P, 512], f16, tag="h2")
                    nc.scalar.activation(out=h2[:, :sz], in_=hp[:, :sz],
                                         func=ACT.Relu, bias=b1d[:, 0:1],
                                         scale=1.0)

                    # three channel segments: a1 (a-ch, first edges),
                    # a2 (a-ch, second edges), b (b-ch, both)
                    segs = (
                        (w2a[0:U, :], h2[0:U, :sz],
                         wka[0:F, :], srcc[0:F, cs], b2a, 0),
                        (w2a[U:2 * U, :], h2[U:2 * U, :sz],
                         wka[F:2 * F, :], srcc[F:2 * F, cs], b2a, 1),
                        (w2bd[:], h2[:, :sz], wkbd[:], srcc[:, cs], b2bd, 2),
                    )
                    for (wew, hrhs, wml, mrhs, b2v, k) in segs:
                        ewp_t = pew.tile([P, 512], f32, space="PSUM", tag="ew")
                        nc.tensor.matmul(out=ewp_t[:, :sz], lhsT=wew,
                                         rhs=hrhs, start=True, stop=True)
                        mlp_t = pml.tile([P, 512], f32, space="PSUM", tag="ml")
                        nc.tensor.matmul(out=mlp_t[:, :sz], lhsT=wml,
                                         rhs=mrhs, start=True, stop=True)
                        ews = ewsp.tile([P, 512], f16, tag="ews")
                        if k == 2 and t % 2 == 1:
                            # balance: odd tiles evacuate the b-segment on DVE
                            nc.vector.tensor_tensor(
                                out=ews[:, :sz], in0=ewp_t[:, :sz],
                                in1=b2v[:, 0:1].broadcast_to([P, sz]),
                                op=ALU.add)
                        else:
                            nc.scalar.activation(
                                out=ews[:, :sz], in_=ewp_t[:, :sz],
                                func=ACT.Identity, bias=b2v[:, 0:1], scale=1.0)
                        # msg = (ew+b2) * ml  (ml read straight from PSUM)
                        nc.vector.tensor_mul(
                            out=msg[:, k * HALF + off:k * HALF + off + sz],
                            in0=ews[:, :sz], in1=mlp_t[:, :sz])
                    off += sz
                    if ci == 0 and pending and g_i == 0:
                        emit_epi(*pending.pop(0))
                # deferred fusion rides after this tile's chunks so each
                # engine FIFO always has ready work queued ahead of it
                if fusq:
                    emit_fusion(*fusq.pop(0))
                fusq.append((t, msg, ohc))
            while fusq:
                emit_fusion(*fusq.pop(0))
            while pending:
                emit_epi(*pending.pop(0))

    nc.compile()
    return nc


# ---------------------------------------------------------------- execution

_CACHE = {}
_LAST_PERM = None


def unpermute(out_raw):
    return out_raw[_LAST_PERM] if _LAST_PERM is not None else out_raw


def _get_nc(cfg):
    key = (cfg.N, cfg.E, cfg.ncores, cfg.NSEG)
    if key not in _CACHE:
        _CACHE[key] = build_nc(cfg)
    return _CACHE[key]


def run(cfg, inputs, trace=False):
    from concourse.bass_utils import run_bass_kernel_spmd
    while True:
        try:
            in_maps = prepare_inputs(cfg, inputs)
            break
        except OverflowError as e:
            cfg = Cfg(cfg.N, cfg.E, cfg.ncores, int(e.args[0]))
    nc = _get_nc(cfg)
    res = run_bass_kernel_spmd(nc, in_maps, list(range(cfg.ncores)),
                               trace=trace)
    out = np.concatenate([r["out"] for r in res.results], axis=0)
    return unpermute(out.astype(np.float32)), res


def kernel(**inputs):
    out, _ = run(FULL, inputs)
    return out


# revision 36
# speedup vs baseline: 1.1583x; 1.1583x over previous
"""EnhancedCGConv GNN message-passing kernel for 8 Trainium2 NeuronCores, v3.

Strategy (dst-sharded, zero collectives, host-side layout prep):
  - Edges are bucketed by destination node tile (128 nodes) per core and
    pair-packed: each SBUF column holds TWO edges (rows 0:64 = first edge,
    64:128 = second edge), enabling block-diagonal weights so every matmul
    streams full-rate.
  - The node-feature gather and the recip-scaled one-hot scatter matrices are
    precomputed on the host and streamed as plain sequential DMA (no gpsimd
    gather, no on-device one-hot generation).
  - Per chunk: PE computes h, ew (192ch), ml (192ch); ACT evacuates ew with
    the b2 bias folded into the copy; DVE forms msg = (ew+b2)*ml reading ml
    straight from PSUM; fusion dense (192->64) is applied per 128-edge
    segment BEFORE the one-hot scatter matmul (linearity of segment-mean);
    the mean's 1/cnt is folded into the one-hot values.
  - Epilogue (fusion bias + group-norm + relu) is batched 8 node tiles at a
    time in SBUF.
"""

import os
import sys

import numpy as np

for _p in ("/opt/trn_rl_repo", "/root/.axon_site/_ro/trn_rl_repo"):
    if os.path.isdir(_p) and _p not in sys.path:
        sys.path.insert(0, _p)

from concourse import bacc, bass, mybir, tile  # noqa: E402

P = 128
F = 64          # node feature dim
U = 64          # units
KK = 3          # num kernels
C = KK * U      # 192
G = 16          # group-norm groups
EPS = 1e-5

f32 = mybir.dt.float32
f16 = mybir.dt.float16
i16 = mybir.dt.int16
NPF16 = mybir.dt.np(f16)


class Cfg:
    def __init__(self, N, E, ncores, NSEG):
        assert NSEG % 2 == 0
        self.N = N
        self.E = E
        self.ncores = ncores
        assert N % ncores == 0
        self.NPC = N // ncores
        self.NT = (self.NPC + P - 1) // P
        self.NSEG = NSEG                  # 128-edge segments per tile
        self.ST = NSEG * P                # edge slots per tile
        self.HALF = self.ST // 2          # pair columns per tile
        # chunk sizes over the pair columns (512 wide + remainder)
        cs = [512] * (self.HALF // 512)
        if self.HALF % 512:
            cs.append(self.HALF % 512)
        self.chunks = cs
        self.EPI = 16                     # node tiles per epilogue batch


FULL = Cfg(N=50000, E=800000, ncores=8, NSEG=16)


def _balance_tiles(deg, NT, cap):
    """Partition nodes into NT tiles (last tile short) so every tile's
    degree sum is <= cap. Returns tile assignment or None if infeasible."""
    NPC = deg.size
    caps = np.full(NT, P, np.int64)
    caps[-1] = NPC - (NT - 1) * P
    order = np.argsort(-deg)
    bsum = np.zeros(NT, np.int64)
    bcnt = np.zeros(NT, np.int64)
    assign = np.empty(NPC, np.int32)
    for node in order:
        avail = np.where(bcnt < caps)[0]
        b = avail[np.argmin(bsum[avail])]
        assign[node] = b
        bsum[b] += deg[node]
        bcnt[b] += 1
    nodes_by_bin = [np.where(assign == b)[0].tolist() for b in range(NT)]

    def swap(hi, lo, i, j):
        a, b2 = nodes_by_bin[hi][i], nodes_by_bin[lo][j]
        nodes_by_bin[hi][i], nodes_by_bin[lo][j] = b2, a
        bsum[hi] += deg[b2] - deg[a]
        bsum[lo] += deg[a] - deg[b2]
        assign[a] = lo
        assign[b2] = hi

    rng = np.random.default_rng(12345)
    for _ in range(200000):
        hi = int(np.argmax(bsum))
        if bsum[hi] <= cap:
            return assign
        need = bsum[hi] - cap
        dh = deg[np.asarray(nodes_by_bin[hi])]
        done = False
        for lo in np.argsort(bsum):
            lo = int(lo)
            if lo == hi:
                continue
            room = cap - bsum[lo]
            if room < need:
                break               # ascending order: no later bin has room
            dl = deg[np.asarray(nodes_by_bin[lo])]
            d = dh[:, None] - dl[None, :]
            ok = (d >= need) & (d <= room)
            if ok.any():
                i, j = np.unravel_index(int(np.argmax(ok)), ok.shape)
                swap(hi, lo, i, j)
                done = True
                break
        if not done:
            # partial progress: move load from hi to a random light bin
            los = np.argsort(bsum)[:8]
            lo = int(rng.choice(los))
            if lo == hi:
                return None
            dl = deg[np.asarray(nodes_by_bin[lo])]
            d = dh[:, None] - dl[None, :]
            limit = bsum[hi] - bsum[lo]
            ok = (d > 0) & (d < limit)
            if not ok.any():
                return None
            dv = np.where(ok, d, -1)
            i, j = np.unravel_index(int(np.argmax(dv)), dv.shape)
            swap(hi, lo, i, j)
    return assign if bsum.max() <= cap else None


def layout(cfg):
    """Section offsets (elements) inside blob16 / blob32."""
    NT, HALF, ST = cfg.NT, cfg.HALF, cfg.ST
    lay = {}
    o = 0
    for name, n in (
        ("ef2", NT * P * HALF),      # [NT][128][HALF] pair-packed edge feats
        ("src2", NT * P * HALF),     # [NT][128][HALF] pair-packed src feats
        ("oh", NT * P * ST),         # [NT][128][NSEG*128] recip-scaled onehot
        ("w1bd", P * P),
        ("w2a", P * P),
        ("w2bd", P * P),
        ("wka", P * P),
        ("wkbd", P * P),
        ("fwa", P * U),
        ("fwb", P * U),              # [fwb; fwb]
    ):
        lay[name] = o
        o += n
    lay["len16"] = o
    o = 0
    for name, n in (
        ("b1d", P),        # [b1; b1]
        ("b2a", P),
        ("b2bd", P),       # [b2b; b2b]
        ("fbb", P * U),    # fusion bias broadcast [128, 64]
        ("gamma", U),
        ("beta", U),
    ):
        lay[name] = o
        o += n
    lay["len32"] = o
    return lay


# ---------------------------------------------------------------- host prep

def prepare_inputs(cfg, inputs):
    nf = np.ascontiguousarray(np.asarray(inputs["node_features"], np.float32))
    ei = np.asarray(inputs["edge_indices"])
    src = ei[0].astype(np.int64)
    dst = ei[1].astype(np.int64)
    ef = np.ascontiguousarray(np.asarray(inputs["edge_features"], np.float32))
    W1 = np.asarray(inputs["edge_W1"], np.float32)
    b1 = np.asarray(inputs["edge_b1"], np.float32)
    W2 = np.asarray(inputs["edge_W2"], np.float32)
    b2 = np.asarray(inputs["edge_b2"], np.float32)
    Wk = np.asarray(inputs["W_kernels"], np.float32)
    fW = np.asarray(inputs["fusion_W"], np.float32)
    fb = np.asarray(inputs["fusion_b"], np.float32)
    gamma = np.asarray(inputs["gamma"], np.float32)
    beta = np.asarray(inputs["beta"], np.float32)

    N, E, NPC, NT = cfg.N, cfg.E, cfg.NPC, cfg.NT
    NSEG, ST, HALF = cfg.NSEG, cfg.ST, cfg.HALF
    ncores = cfg.ncores
    lay = layout(cfg)

    core = dst // NPC
    loc = dst - core * NPC

    # degree-balanced node->tile permutation per core (minimizes NSEG)
    deg_all = np.bincount(dst, minlength=N)
    permpos = np.empty(N, np.int64)       # original node -> permuted row
    for c in range(ncores):
        dg = deg_all[c * NPC:(c + 1) * NPC].astype(np.int64)
        assign = _balance_tiles(dg, NT, ST)
        if assign is None:
            raise OverflowError(NSEG + 2)
        pp = np.empty(NPC, np.int64)
        order_n = np.argsort(assign, kind="stable")
        ofs = np.zeros(NT + 1, np.int64)
        np.cumsum(np.bincount(assign, minlength=NT), out=ofs[1:])
        ranks = np.arange(NPC) - ofs[assign[order_n]]
        pp[order_n] = assign[order_n] * P + ranks
        permpos[c * NPC:(c + 1) * NPC] = pp
    global _LAST_PERM
    # original global node -> row in the raw (permuted) device output
    _LAST_PERM = (np.arange(N) // NPC) * NPC + permpos

    ploc = permpos[dst]                   # permuted position within core
    t = ploc >> 7
    n_in_tile = ploc & 127
    key = core * NT + t                      # global tile id
    ntile = ncores * NT
    cnt = np.bincount(key, minlength=ntile)
    if cnt.max() > ST:
        raise OverflowError(NSEG + 2)

    order = np.argsort(key, kind="stable")
    starts = np.zeros(ntile + 1, np.int64)
    np.cumsum(cnt, out=starts[1:])
    rank = np.arange(E, dtype=np.int64) - starts[key[order]]
    ko = key[order]

    # slot arrays [ntile * ST]
    slot = ko * ST + rank
    TOT = ntile * ST
    ef_s = np.zeros((TOT, F), np.float32)
    ef_s[slot] = ef[order]
    src_s = np.zeros((TOT, F), np.float32)
    src_s[slot] = nf[src[order]]

    # recip-scaled one-hot [ntile, 128, NSEG*128] fp16
    cntN = np.bincount(dst, minlength=N).astype(np.float32)
    recip = (1.0 / np.maximum(cntN, 1.0)).astype(NPF16)
    oh = np.zeros(ntile * P * ST, NPF16)
    seg = rank >> 7
    i_in_seg = rank & 127
    flat = (ko * P + i_in_seg) * ST + seg * P + n_in_tile[order]
    oh[flat] = recip[dst[order]]
    oh = oh.view(np.int16).reshape(ntile, P, ST)

    def pack_pairs(xs):
        # [ntile*ST, F] -> [ntile, 128 rows (2xF), HALF cols]
        a = xs.reshape(ntile, NSEG // 2, 2, P, F)
        return np.ascontiguousarray(
            a.transpose(0, 2, 4, 1, 3).astype(NPF16).view(np.int16)
            .reshape(ntile, 2 * F, HALF))

    ef2 = pack_pairs(ef_s)
    src2 = pack_pairs(src_s)

    W_all = np.ascontiguousarray(Wk.transpose(1, 0, 2).reshape(F, C))

    def f16i(x):
        return np.ascontiguousarray(x, np.float32).astype(NPF16).view(np.int16)

    def blockdiag(w):
        z = np.zeros((2 * F, 2 * w.shape[1]), np.float32)
        z[:F, :w.shape[1]] = w
        z[F:, w.shape[1]:] = w
        return z

    w1bd = f16i(blockdiag(W1))
    w2a = f16i(np.concatenate([W2[:, :P], W2[:, :P]], axis=0))
    w2bd = f16i(blockdiag(W2[:, P:]))
    wka = f16i(np.concatenate([W_all[:, :P], W_all[:, :P]], axis=0))
    wkbd = f16i(blockdiag(W_all[:, P:]))
    fwa = f16i(fW[:P])
    fwb = f16i(np.concatenate([fW[P:], fW[P:]], axis=0))

    blob32_tail = np.concatenate([
        np.tile(b1, 2), b2[:P], np.tile(b2[P:], 2),
        np.tile(fb, P), gamma, beta]).astype(np.float32)

    in_maps = []
    for c in range(ncores):
        sl = slice(c * NT, (c + 1) * NT)
        blob16 = np.concatenate([
            ef2[sl].ravel(), src2[sl].ravel(), oh[sl].ravel(),
            w1bd.ravel(), w2a.ravel(), w2bd.ravel(),
            wka.ravel(), wkbd.ravel(), fwa.ravel(), fwb.ravel()])
        assert blob16.size == lay["len16"], (blob16.size, lay["len16"])
        assert blob32_tail.size == lay["len32"]
        in_maps.append({"blob16": blob16, "blob32": blob32_tail})
    return in_maps


# ---------------------------------------------------------------- device IR

def build_nc(cfg, nt_limit=None):
    NPC, NT, NSEG = cfg.NPC, cfg.NT, cfg.NSEG
    ST, HALF, EPI = cfg.ST, cfg.HALF, cfg.EPI
    lay = layout(cfg)
    nc = bacc.Bacc("TRN2", target_bir_lowering=False)

    d16 = nc.dram_tensor("blob16", [lay["len16"]], i16, kind="ExternalInput")
    d32 = nc.dram_tensor("blob32", [lay["len32"]], f32, kind="ExternalInput")
    d_out = nc.dram_tensor("out", [NPC, U], f32, kind="ExternalOutput")

    ACT = mybir.ActivationFunctionType
    ALU = mybir.AluOpType
    AX = mybir.AxisListType

    def v16(name, rows, cols, extra_off=0, rowstride=None):
        return bass.AP(
            tensor=d16, offset=lay[name] + extra_off,
            ap=[[rowstride if rowstride is not None else cols, rows],
                [1, cols]]).bitcast(f16)

    def v32(name, rows, cols, extra_off=0):
        return bass.AP(tensor=d32, offset=lay[name] + extra_off,
                       ap=[[cols, rows], [1, cols]])

    NTS = NT if nt_limit is None else min(NT, nt_limit)

    with tile.TileContext(nc) as tc:
        with tc.tile_pool(name="const", bufs=1) as const, \
             tc.tile_pool(name="efp", bufs=6) as efp, \
             tc.tile_pool(name="srcp", bufs=6) as srcp, \
             tc.tile_pool(name="ohp", bufs=4) as ohp, \
             tc.tile_pool(name="h2p", bufs=4) as h2p, \
             tc.tile_pool(name="ewsp", bufs=6) as ewsp, \
             tc.tile_pool(name="msgp", bufs=3) as msgp, \
             tc.tile_pool(name="fsp", bufs=6) as fsp, \
             tc.tile_pool(name="epi", bufs=2) as epi, \
             tc.tile_pool(name="pscr", bufs=2, space="PSUM") as pscr, \
             tc.tile_pool(name="pew", bufs=2, space="PSUM") as pew, \
             tc.tile_pool(name="pml", bufs=3, space="PSUM") as pml, \
             tc.tile_pool(name="pacc", bufs=1, space="PSUM") as pacc:

            # ---------------- constants ----------------
            def load16(name, rows, cols):
                t_ = const.tile([rows, cols], f16, tag=name)
                nc.sync.dma_start(out=t_[:], in_=v16(name, rows, cols))
                return t_

            w1bd = load16("w1bd", P, P)
            w2a = load16("w2a", P, P)
            w2bd = load16("w2bd", P, P)
            wka = load16("wka", P, P)
            wkbd = load16("wkbd", P, P)
            fwa = load16("fwa", P, U)
            fwb = load16("fwb", P, U)

            def load32(name, rows, cols):
                t_ = const.tile([rows, cols], f32, tag=name)
                nc.sync.dma_start(out=t_[:], in_=v32(name, rows, cols))
                return t_

            b1d = load32("b1d", P, 1)
            b2a = load32("b2a", P, 1)
            b2bd = load32("b2bd", P, 1)
            fbb = load32("fbb", P, U)

            def bcast32(name, w):
                # [P, w*U]: per-channel vector tiled w times along free dim
                t_ = const.tile([P, w * U], f32, tag=name + "b")
                nc.gpsimd.dma_start(
                    out=t_[:],
                    in_=bass.AP(tensor=d32, offset=lay[name],
                                ap=[[0, P], [0, w], [1, U]]))
                return t_

            gammab = bcast32("gamma", EPI)
            betab = bcast32("beta", EPI)
            epsb = const.tile([P, 1], f32)
            nc.vector.memset(epsb[:], EPS)

            # ---------------- epilogue emitter (deferred) ----------------
            pending = []

            def emit_epi(t0, g_n, fu32):
                W = g_n * U
                NG = g_n * G
                D = U // G
                fu3 = fu32[:, :W].rearrange("p (g d) -> p g d", g=NG)
                sumg = epi.tile([P, EPI * G], f32, tag="sumg")
                nc.vector.tensor_reduce(
                    out=sumg[:, :NG], in_=fu3, axis=AX.X, op=ALU.add)
                xsq = epi.tile([P, EPI * U], f32, tag="xsq")
                nc.scalar.activation(out=xsq[:, :W], in_=fu32[:, :W],
                                     func=ACT.Square)
                sqg = epi.tile([P, EPI * G], f32, tag="sqg")
                nc.vector.tensor_reduce(
                    out=sqg[:, :NG],
                    in_=xsq[:, :W].rearrange("p (g d) -> p g d", g=NG),
                    axis=AX.X, op=ALU.add)
                # var = sqg/D - (sumg/D)^2 ; rstd = 1/sqrt(var + eps)
                tmp = epi.tile([P, EPI * G], f32, tag="tmp")
                nc.vector.scalar_tensor_tensor(
                    out=tmp[:, :NG], in0=sumg[:, :NG], scalar=1.0 / (D * D),
                    in1=sumg[:, :NG], op0=ALU.mult, op1=ALU.mult)
                varg = epi.tile([P, EPI * G], f32, tag="varg")
                nc.vector.scalar_tensor_tensor(
                    out=varg[:, :NG], in0=sqg[:, :NG], scalar=1.0 / D,
                    in1=tmp[:, :NG], op0=ALU.mult, op1=ALU.subtract)
                stdg = epi.tile([P, EPI * G], f32, tag="stdg")
                nc.scalar.activation(out=stdg[:, :NG], in_=varg[:, :NG],
                                     func=ACT.Sqrt, bias=epsb[:, 0:1],
                                     scale=1.0)
                rstd = epi.tile([P, EPI * G], f32, tag="rstd")
                nc.vector.reciprocal(out=rstd[:, :NG], in_=stdg[:, :NG])
                mr = epi.tile([P, EPI * G], f32, tag="mr")
                nc.vector.scalar_tensor_tensor(
                    out=mr[:, :NG], in0=sumg[:, :NG], scalar=1.0 / D,
                    in1=rstd[:, :NG], op0=ALU.mult, op1=ALU.mult)
                xn = epi.tile([P, EPI * U], f32, tag="xn")
                xn3 = xn[:, :W].rearrange("p (g d) -> p g d", g=NG)
                rstdb = rstd[:, :NG].unsqueeze(2).broadcast_to([P, NG, D])
                mrb = mr[:, :NG].unsqueeze(2).broadcast_to([P, NG, D])
                nc.vector.tensor_mul(out=xn3, in0=fu3, in1=rstdb)
                nc.vector.tensor_sub(out=xn3, in0=xn3, in1=mrb)
                nc.vector.tensor_mul(out=xn[:, :W], in0=xn[:, :W],
                                     in1=gammab[:, :W])
                nc.vector.tensor_add(out=xn[:, :W], in0=xn[:, :W],
                                     in1=betab[:, :W])
                outf = epi.tile([P, EPI * U], f32, tag="outf")
                nc.scalar.activation(out=outf[:, :W], in_=xn[:, :W],
                                     func=ACT.Relu)
                rows_all = min(g_n * P, NPC - t0 * P)
                full_g = rows_all // P
                if full_g:
                    nc.sync.dma_start(
                        out=bass.AP(tensor=d_out, offset=t0 * P * U,
                                    ap=[[U, P], [P * U, full_g], [1, U]]),
                        in_=outf[:, :full_g * U].rearrange(
                            "p (g u) -> p g u", u=U))
                rem = rows_all - full_g * P
                if rem > 0:
                    nc.sync.dma_start(
                        out=bass.AP(tensor=d_out,
                                    offset=(t0 + full_g) * P * U,
                                    ap=[[U, rem], [1, U]]),
                        in_=outf[:rem, full_g * U:(full_g + 1) * U])

            # ------------- fusion + scatter emitter (deferred) -------------
            fstate = {"acc8": None, "fu32": None}

            def emit_fusion(t, msg, ohc):
                g_i = t % EPI
                g_n = min(EPI, NTS - (t - g_i))
                o_i = t % 8
                if o_i == 0:
                    fstate["acc8"] = pacc.tile([P, 512], f32, space="PSUM",
                                               tag="acc", name="acc8")
                acc8 = fstate["acc8"]
                acc = acc8[:, o_i * U:(o_i + 1) * U]
                for r0 in range(0, NSEG, 8):          # rounds of 8 segments
                    rn = min(8, NSEG - r0)
                    f6 = pscr.tile([P, 512], f32, space="PSUM", tag="scr")
                    for k in range(rn):
                        s = r0 + k
                        jj = s >> 1
                        h = s & 1
                        fo = slice(k * U, (k + 1) * U)
                        nc.tensor.matmul(
                            out=f6[:, fo],
                            lhsT=msg[:, h * HALF + jj * P:
                                     h * HALF + (jj + 1) * P],
                            rhs=fwa[:], start=True, stop=False)
                        nc.tensor.matmul(
                            out=f6[:, fo],
                            lhsT=msg[h * U:(h + 1) * U,
                                     2 * HALF + jj * P:2 * HALF + (jj + 1) * P],
                            rhs=fwb[h * U:(h + 1) * U, :], start=False,
                            stop=True)
                    fsb = fsp.tile([P, 512], f16, tag="fsb")
                    nc.scalar.activation(out=fsb[:, :rn * U],
                                         in_=f6[:, :rn * U], func=ACT.Copy)
                    for k in range(rn):
                        s = r0 + k
                        nc.tensor.matmul(
                            out=acc, lhsT=ohc[:, s * P:(s + 1) * P],
                            rhs=fsb[:, k * U:(k + 1) * U],
                            start=(s == 0), stop=(s == NSEG - 1))

                # acc evacuation (adds fusion bias)
                if g_i == 0:
                    fstate["fu32"] = epi.tile([P, EPI * U], f32, tag="fu32",
                                              name="fu32")
                fu32 = fstate["fu32"]
                if o_i == 7 or t == NTS - 1:
                    na = o_i + 1
                    nc.vector.tensor_add(
                        out=fu32[:].rearrange("p (g u) -> p g u", u=U)
                            [:, g_i - o_i:g_i + 1, :],
                        in0=acc8[:].rearrange("p (g u) -> p g u", u=U)
                            [:, 0:na, :],
                        in1=fbb[:].unsqueeze(1).broadcast_to([P, na, U]))
                if g_i == g_n - 1:
                    pending.append((t - g_i, g_n, fu32))

            # ---------------- main loop ----------------
            fusq = []
            for t in range(NTS):
                g_i = t % EPI            # index within epilogue batch

                ef2c = efp.tile([P, HALF], f16, tag="ef2")
                nc.sync.dma_start(
                    out=ef2c[:], in_=v16("ef2", P, HALF, extra_off=t * P * HALF))
                srcc = srcp.tile([P, HALF], f16, tag="src2")
                nc.sync.dma_start(
                    out=srcc[:], in_=v16("src2", P, HALF, extra_off=t * P * HALF))
                ohc = ohp.tile([P, ST], f16, tag="oh")
                nc.sync.dma_start(
                    out=ohc[:], in_=v16("oh", P, ST, extra_off=t * P * ST))

                msg = msgp.tile([P, 3 * HALF], f16, tag="msg")
                off = 0
                for ci, sz in enumerate(cfg.chunks):
                    cs = slice(off, off + sz)
                    hp = pscr.tile([P, 512], f32, space="PSUM", tag="scr")
                    nc.tensor.matmul(out=hp[:, :sz], lhsT=w1bd[:],
                                     rhs=ef2c[:, cs], start=True, stop=True)
                    h2 = h2p.tile([P, 512], f16, tag="h2")
                    nc.scalar.activation(out=h2[:, :sz], in_=hp[:, :sz],
                                         func=ACT.Relu, bias=b1d[:, 0:1],
                                         scale=1.0)

                    # three channel segments: a1 (a-ch, first edges),
                    # a2 (a-ch, second edges), b (b-ch, both)
                    segs = (
                        (w2a[0:U, :], h2[0:U, :sz],
                         wka[0:F, :], srcc[0:F, cs], b2a, 0),
                        (w2a[U:2 * U, :], h2[U:2 * U, :sz],
                         wka[F:2 * F, :], srcc[F:2 * F, cs], b2a, 1),
                        (w2bd[:], h2[:, :sz], wkbd[:], srcc[:, cs], b2bd, 2),
                    )
                    for (wew, hrhs, wml, mrhs, b2v, k) in segs:
                        ewp_t = pew.tile([P, 512], f32, space="PSUM", tag="ew")
                        nc.tensor.matmul(out=ewp_t[:, :sz], lhsT=wew,
                                         rhs=hrhs, start=True, stop=True)
                        mlp_t = pml.tile([P, 512], f32, space="PSUM", tag="ml")
                        nc.tensor.matmul(out=mlp_t[:, :sz], lhsT=wml,
                                         rhs=mrhs, start=True, stop=True)
                        ews = ewsp.tile([P, 512], f16, tag="ews")
                        if t % 2 == 1 and (k == 2 or (k == 1 and ci == 0)):
                            # balance: odd tiles evacuate the b-segment on DVE
                            nc.vector.tensor_tensor(
                                out=ews[:, :sz], in0=ewp_t[:, :sz],
                                in1=b2v[:, 0:1].broadcast_to([P, sz]),
                                op=ALU.add)
                        else:
                            nc.scalar.activation(
                                out=ews[:, :sz], in_=ewp_t[:, :sz],
                                func=ACT.Identity, bias=b2v[:, 0:1], scale=1.0)
                        # msg = (ew+b2) * ml  (ml read straight from PSUM)
                        nc.vector.tensor_mul(
                            out=msg[:, k * HALF + off:k * HALF + off + sz],
                            in0=ews[:, :sz], in1=mlp_t[:, :sz])
                    off += sz
                    if ci == 0 and pending and g_i == 0:
                        emit_epi(*pending.pop(0))
                # deferred fusion rides after this tile's chunks so each
                # engine FIFO always has ready work queued ahead of it
                if fusq:
                    emit_fusion(*fusq.pop(0))
                fusq.append((t, msg, ohc))
            while fusq:
                emit_fusion(*fusq.pop(0))
            while pending:
                emit_epi(*pending.pop(0))

    nc.compile()
    return nc


# ---------------------------------------------------------------- execution

_CACHE = {}
_LAST_PERM = None


def unpermute(out_raw):
    return out_raw[_LAST_PERM] if _LAST_PERM is not None else out_raw


def _get_nc(cfg):
    key = (cfg.N, cfg.E, cfg.ncores, cfg.NSEG)
    if key not in _CACHE:
        _CACHE[key] = build_nc(cfg)
    return _CACHE[key]


def run(cfg, inputs, trace=False):
    from concourse.bass_utils import run_bass_kernel_spmd
    while True:
        try:
            in_maps = prepare_inputs(cfg, inputs)
            break
        except OverflowError as e:
            cfg = Cfg(cfg.N, cfg.E, cfg.ncores, int(e.args[0]))
    nc = _get_nc(cfg)
    res = run_bass_kernel_spmd(nc, in_maps, list(range(cfg.ncores)),
                               trace=trace)
    out = np.concatenate([r["out"] for r in res.results], axis=0)
    return unpermute(out.astype(np.float32)), res


def kernel(**inputs):
    out, _ = run(FULL, inputs)
    return out


# revision 38
# speedup vs baseline: 1.1666x; 1.0072x over previous
"""EnhancedCGConv GNN message-passing kernel for 8 Trainium2 NeuronCores, v3.

Strategy (dst-sharded, zero collectives, host-side layout prep):
  - Edges are bucketed by destination node tile (128 nodes) per core and
    pair-packed: each SBUF column holds TWO edges (rows 0:64 = first edge,
    64:128 = second edge), enabling block-diagonal weights so every matmul
    streams full-rate.
  - The node-feature gather and the recip-scaled one-hot scatter matrices are
    precomputed on the host and streamed as plain sequential DMA (no gpsimd
    gather, no on-device one-hot generation).
  - Per chunk: PE computes h, ew (192ch), ml (192ch); ACT evacuates ew with
    the b2 bias folded into the copy; DVE forms msg = (ew+b2)*ml reading ml
    straight from PSUM; fusion dense (192->64) is applied per 128-edge
    segment BEFORE the one-hot scatter matmul (linearity of segment-mean);
    the mean's 1/cnt is folded into the one-hot values.
  - Epilogue (fusion bias + group-norm + relu) is batched 8 node tiles at a
    time in SBUF.
"""

import os
import sys

import numpy as np

for _p in ("/opt/trn_rl_repo", "/root/.axon_site/_ro/trn_rl_repo"):
    if os.path.isdir(_p) and _p not in sys.path:
        sys.path.insert(0, _p)

from concourse import bacc, bass, mybir, tile  # noqa: E402

P = 128
F = 64          # node feature dim
U = 64          # units
KK = 3          # num kernels
C = KK * U      # 192
G = 16          # group-norm groups
EPS = 1e-5

f32 = mybir.dt.float32
f16 = mybir.dt.float16
i16 = mybir.dt.int16
NPF16 = mybir.dt.np(f16)


class Cfg:
    def __init__(self, N, E, ncores, NSEG):
        assert NSEG % 2 == 0
        self.N = N
        self.E = E
        self.ncores = ncores
        assert N % ncores == 0
        self.NPC = N // ncores
        self.NT = (self.NPC + P - 1) // P
        self.NSEG = NSEG                  # 128-edge segments per tile
        self.ST = NSEG * P                # edge slots per tile
        self.HALF = self.ST // 2          # pair columns per tile
        # chunk sizes over the pair columns (512 wide + remainder)
        cs = [512] * (self.HALF // 512)
        if self.HALF % 512:
            cs.append(self.HALF % 512)
        self.chunks = cs
        self.EPI = 16                     # node tiles per epilogue batch


FULL = Cfg(N=50000, E=800000, ncores=8, NSEG=16)


def _balance_tiles(deg, NT, cap):
    """Partition nodes into NT tiles (last tile short) so every tile's
    degree sum is <= cap. Returns tile assignment or None if infeasible."""
    NPC = deg.size
    caps = np.full(NT, P, np.int64)
    caps[-1] = NPC - (NT - 1) * P
    order = np.argsort(-deg)
    bsum = np.zeros(NT, np.int64)
    bcnt = np.zeros(NT, np.int64)
    assign = np.empty(NPC, np.int32)
    for node in order:
        avail = np.where(bcnt < caps)[0]
        b = avail[np.argmin(bsum[avail])]
        assign[node] = b
        bsum[b] += deg[node]
        bcnt[b] += 1
    nodes_by_bin = [np.where(assign == b)[0].tolist() for b in range(NT)]

    def swap(hi, lo, i, j):
        a, b2 = nodes_by_bin[hi][i], nodes_by_bin[lo][j]
        nodes_by_bin[hi][i], nodes_by_bin[lo][j] = b2, a
        bsum[hi] += deg[b2] - deg[a]
        bsum[lo] += deg[a] - deg[b2]
        assign[a] = lo
        assign[b2] = hi

    rng = np.random.default_rng(12345)
    for _ in range(200000):
        hi = int(np.argmax(bsum))
        if bsum[hi] <= cap:
            return assign
        need = bsum[hi] - cap
        dh = deg[np.asarray(nodes_by_bin[hi])]
        done = False
        for lo in np.argsort(bsum):
            lo = int(lo)
            if lo == hi:
                continue
            room = cap - bsum[lo]
            if room < need:
                break               # ascending order: no later bin has room
            dl = deg[np.asarray(nodes_by_bin[lo])]
            d = dh[:, None] - dl[None, :]
            ok = (d >= need) & (d <= room)
            if ok.any():
                i, j = np.unravel_index(int(np.argmax(ok)), ok.shape)
                swap(hi, lo, i, j)
                done = True
                break
        if not done:
            # partial progress: move load from hi to a random light bin
            los = np.argsort(bsum)[:8]
            lo = int(rng.choice(los))
            if lo == hi:
                return None
            dl = deg[np.asarray(nodes_by_bin[lo])]
            d = dh[:, None] - dl[None, :]
            limit = bsum[hi] - bsum[lo]
            ok = (d > 0) & (d < limit)
            if not ok.any():
                return None
            dv = np.where(ok, d, -1)
            i, j = np.unravel_index(int(np.argmax(dv)), dv.shape)
            swap(hi, lo, i, j)
    return assign if bsum.max() <= cap else None


def layout(cfg):
    """Section offsets (elements) inside blob16 / blob32."""
    NT, HALF, ST = cfg.NT, cfg.HALF, cfg.ST
    lay = {}
    o = 0
    for name, n in (
        ("ef2", NT * P * HALF),      # [NT][128][HALF] pair-packed edge feats
        ("src2", NT * P * HALF),     # [NT][128][HALF] pair-packed src feats
        ("oh", NT * P * ST),         # [NT][128][NSEG*128] recip-scaled onehot
        ("w1bd", P * P),
        ("w2a", P * P),
        ("w2bd", P * P),
        ("wka", P * P),
        ("wkbd", P * P),
        ("fwa", P * U),
        ("fwb", P * U),              # [fwb; fwb]
    ):
        lay[name] = o
        o += n
    lay["len16"] = o
    o = 0
    for name, n in (
        ("b1d", P),        # [b1; b1]
        ("b2a", P),
        ("b2bd", P),       # [b2b; b2b]
        ("fbb", P * U),    # fusion bias broadcast [128, 64]
        ("gamma", U),
        ("beta", U),
    ):
        lay[name] = o
        o += n
    lay["len32"] = o
    return lay


# ---------------------------------------------------------------- host prep

def prepare_inputs(cfg, inputs):
    nf = np.ascontiguousarray(np.asarray(inputs["node_features"], np.float32))
    ei = np.asarray(inputs["edge_indices"])
    src = ei[0].astype(np.int64)
    dst = ei[1].astype(np.int64)
    ef = np.ascontiguousarray(np.asarray(inputs["edge_features"], np.float32))
    W1 = np.asarray(inputs["edge_W1"], np.float32)
    b1 = np.asarray(inputs["edge_b1"], np.float32)
    W2 = np.asarray(inputs["edge_W2"], np.float32)
    b2 = np.asarray(inputs["edge_b2"], np.float32)
    Wk = np.asarray(inputs["W_kernels"], np.float32)
    fW = np.asarray(inputs["fusion_W"], np.float32)
    fb = np.asarray(inputs["fusion_b"], np.float32)
    gamma = np.asarray(inputs["gamma"], np.float32)
    beta = np.asarray(inputs["beta"], np.float32)

    N, E, NPC, NT = cfg.N, cfg.E, cfg.NPC, cfg.NT
    NSEG, ST, HALF = cfg.NSEG, cfg.ST, cfg.HALF
    ncores = cfg.ncores
    lay = layout(cfg)

    core = dst // NPC
    loc = dst - core * NPC

    # degree-balanced node->tile permutation per core (minimizes NSEG)
    deg_all = np.bincount(dst, minlength=N)
    permpos = np.empty(N, np.int64)       # original node -> permuted row
    for c in range(ncores):
        dg = deg_all[c * NPC:(c + 1) * NPC].astype(np.int64)
        assign = _balance_tiles(dg, NT, ST)
        if assign is None:
            raise OverflowError(NSEG + 2)
        pp = np.empty(NPC, np.int64)
        order_n = np.argsort(assign, kind="stable")
        ofs = np.zeros(NT + 1, np.int64)
        np.cumsum(np.bincount(assign, minlength=NT), out=ofs[1:])
        ranks = np.arange(NPC) - ofs[assign[order_n]]
        pp[order_n] = assign[order_n] * P + ranks
        permpos[c * NPC:(c + 1) * NPC] = pp
    global _LAST_PERM
    # original global node -> row in the raw (permuted) device output
    _LAST_PERM = (np.arange(N) // NPC) * NPC + permpos

    ploc = permpos[dst]                   # permuted position within core
    t = ploc >> 7
    n_in_tile = ploc & 127
    key = core * NT + t                      # global tile id
    ntile = ncores * NT
    cnt = np.bincount(key, minlength=ntile)
    if cnt.max() > ST:
        raise OverflowError(NSEG + 2)

    order = np.argsort(key, kind="stable")
    starts = np.zeros(ntile + 1, np.int64)
    np.cumsum(cnt, out=starts[1:])
    rank = np.arange(E, dtype=np.int64) - starts[key[order]]
    ko = key[order]

    # slot arrays [ntile * ST]
    slot = ko * ST + rank
    TOT = ntile * ST
    ef_s = np.zeros((TOT, F), np.float32)
    ef_s[slot] = ef[order]
    src_s = np.zeros((TOT, F), np.float32)
    src_s[slot] = nf[src[order]]

    # recip-scaled one-hot [ntile, 128, NSEG*128] fp16
    cntN = np.bincount(dst, minlength=N).astype(np.float32)
    recip = (1.0 / np.maximum(cntN, 1.0)).astype(NPF16)
    oh = np.zeros(ntile * P * ST, NPF16)
    seg = rank >> 7
    i_in_seg = rank & 127
    flat = (ko * P + i_in_seg) * ST + seg * P + n_in_tile[order]
    oh[flat] = recip[dst[order]]
    oh = oh.view(np.int16).reshape(ntile, P, ST)

    def pack_pairs(xs):
        # [ntile*ST, F] -> [ntile, 128 rows (2xF), HALF cols]
        a = xs.reshape(ntile, NSEG // 2, 2, P, F)
        return np.ascontiguousarray(
            a.transpose(0, 2, 4, 1, 3).astype(NPF16).view(np.int16)
            .reshape(ntile, 2 * F, HALF))

    ef2 = pack_pairs(ef_s)
    src2 = pack_pairs(src_s)

    W_all = np.ascontiguousarray(Wk.transpose(1, 0, 2).reshape(F, C))

    def f16i(x):
        return np.ascontiguousarray(x, np.float32).astype(NPF16).view(np.int16)

    def blockdiag(w):
        z = np.zeros((2 * F, 2 * w.shape[1]), np.float32)
        z[:F, :w.shape[1]] = w
        z[F:, w.shape[1]:] = w
        return z

    w1bd = f16i(blockdiag(W1))
    w2a = f16i(np.concatenate([W2[:, :P], W2[:, :P]], axis=0))
    w2bd = f16i(blockdiag(W2[:, P:]))
    wka = f16i(np.concatenate([W_all[:, :P], W_all[:, :P]], axis=0))
    wkbd = f16i(blockdiag(W_all[:, P:]))
    fwa = f16i(fW[:P])
    fwb = f16i(np.concatenate([fW[P:], fW[P:]], axis=0))

    blob32_tail = np.concatenate([
        np.tile(b1, 2), b2[:P], np.tile(b2[P:], 2),
        np.tile(fb, P), gamma, beta]).astype(np.float32)

    in_maps = []
    for c in range(ncores):
        sl = slice(c * NT, (c + 1) * NT)
        blob16 = np.concatenate([
            ef2[sl].ravel(), src2[sl].ravel(), oh[sl].ravel(),
            w1bd.ravel(), w2a.ravel(), w2bd.ravel(),
            wka.ravel(), wkbd.ravel(), fwa.ravel(), fwb.ravel()])
        assert blob16.size == lay["len16"], (blob16.size, lay["len16"])
        assert blob32_tail.size == lay["len32"]
        in_maps.append({"blob16": blob16, "blob32": blob32_tail})
    return in_maps


# ---------------------------------------------------------------- device IR

def build_nc(cfg, nt_limit=None):
    NPC, NT, NSEG = cfg.NPC, cfg.NT, cfg.NSEG
    ST, HALF, EPI = cfg.ST, cfg.HALF, cfg.EPI
    lay = layout(cfg)
    nc = bacc.Bacc("TRN2", target_bir_lowering=False)

    d16 = nc.dram_tensor("blob16", [lay["len16"]], i16, kind="ExternalInput")
    d32 = nc.dram_tensor("blob32", [lay["len32"]], f32, kind="ExternalInput")
    d_out = nc.dram_tensor("out", [NPC, U], f32, kind="ExternalOutput")

    ACT = mybir.ActivationFunctionType
    ALU = mybir.AluOpType
    AX = mybir.AxisListType

    def v16(name, rows, cols, extra_off=0, rowstride=None):
        return bass.AP(
            tensor=d16, offset=lay[name] + extra_off,
            ap=[[rowstride if rowstride is not None else cols, rows],
                [1, cols]]).bitcast(f16)

    def v32(name, rows, cols, extra_off=0):
        return bass.AP(tensor=d32, offset=lay[name] + extra_off,
                       ap=[[cols, rows], [1, cols]])

    NTS = NT if nt_limit is None else min(NT, nt_limit)

    with tile.TileContext(nc) as tc:
        with tc.tile_pool(name="const", bufs=1) as const, \
             tc.tile_pool(name="efp", bufs=6) as efp, \
             tc.tile_pool(name="srcp", bufs=6) as srcp, \
             tc.tile_pool(name="ohp", bufs=4) as ohp, \
             tc.tile_pool(name="h2p", bufs=4) as h2p, \
             tc.tile_pool(name="ewsp", bufs=6) as ewsp, \
             tc.tile_pool(name="msgp", bufs=3) as msgp, \
             tc.tile_pool(name="fsp", bufs=6) as fsp, \
             tc.tile_pool(name="epi", bufs=2) as epi, \
             tc.tile_pool(name="pscr", bufs=2, space="PSUM") as pscr, \
             tc.tile_pool(name="pew", bufs=2, space="PSUM") as pew, \
             tc.tile_pool(name="pml", bufs=3, space="PSUM") as pml, \
             tc.tile_pool(name="pacc", bufs=1, space="PSUM") as pacc:

            # ---------------- constants + tile-0 prefetch ----------------
            def tile_dmas(t):
                ef2c = efp.tile([P, HALF], f16, tag="ef2", name="ef2c")
                nc.sync.dma_start(
                    out=ef2c[:],
                    in_=v16("ef2", P, HALF, extra_off=t * P * HALF))
                srcc = srcp.tile([P, HALF], f16, tag="src2", name="srcc")
                nc.sync.dma_start(
                    out=srcc[:],
                    in_=v16("src2", P, HALF, extra_off=t * P * HALF))
                return ef2c, srcc

            def oh_dma(t):
                ohc = ohp.tile([P, ST], f16, tag="oh", name="ohc")
                nc.sync.dma_start(
                    out=ohc[:], in_=v16("oh", P, ST, extra_off=t * P * ST))
                return ohc

            def load16(name, rows, cols):
                t_ = const.tile([rows, cols], f16, tag=name)
                nc.sync.dma_start(out=t_[:], in_=v16(name, rows, cols))
                return t_

            def load32(name, rows, cols):
                t_ = const.tile([rows, cols], f32, tag=name)
                nc.sync.dma_start(out=t_[:], in_=v32(name, rows, cols))
                return t_

            # tile-0 data first so the pipeline fills while the SP sequencer
            # issues the remaining constant loads
            pre0 = tile_dmas(0)
            w1bd = load16("w1bd", P, P)
            b1d = load32("b1d", P, 1)
            wka = load16("wka", P, P)
            w2a = load16("w2a", P, P)
            w2bd = load16("w2bd", P, P)
            wkbd = load16("wkbd", P, P)
            b2a = load32("b2a", P, 1)
            b2bd = load32("b2bd", P, 1)
            pre0 = pre0 + (oh_dma(0),)
            fwa = load16("fwa", P, U)
            fwb = load16("fwb", P, U)
            fbb = load32("fbb", P, U)

            def bcast32(name, w):
                # [P, w*U]: per-channel vector tiled w times along free dim
                t_ = const.tile([P, w * U], f32, tag=name + "b")
                nc.gpsimd.dma_start(
                    out=t_[:],
                    in_=bass.AP(tensor=d32, offset=lay[name],
                                ap=[[0, P], [0, w], [1, U]]))
                return t_

            gammab = bcast32("gamma", EPI)
            betab = bcast32("beta", EPI)
            epsb = const.tile([P, 1], f32)
            nc.vector.memset(epsb[:], EPS)

            # ---------------- epilogue emitter (deferred) ----------------
            pending = []

            def emit_epi(t0, g_n, fu32):
                W = g_n * U
                NG = g_n * G
                D = U // G
                fu3 = fu32[:, :W].rearrange("p (g d) -> p g d", g=NG)
                sumg = epi.tile([P, EPI * G], f32, tag="sumg")
                nc.vector.tensor_reduce(
                    out=sumg[:, :NG], in_=fu3, axis=AX.X, op=ALU.add)
                xsq = epi.tile([P, EPI * U], f32, tag="xsq")
                nc.scalar.activation(out=xsq[:, :W], in_=fu32[:, :W],
                                     func=ACT.Square)
                sqg = epi.tile([P, EPI * G], f32, tag="sqg")
                nc.vector.tensor_reduce(
                    out=sqg[:, :NG],
                    in_=xsq[:, :W].rearrange("p (g d) -> p g d", g=NG),
                    axis=AX.X, op=ALU.add)
                # var = sqg/D - (sumg/D)^2 ; rstd = 1/sqrt(var + eps)
                tmp = epi.tile([P, EPI * G], f32, tag="tmp")
                nc.vector.scalar_tensor_tensor(
                    out=tmp[:, :NG], in0=sumg[:, :NG], scalar=1.0 / (D * D),
                    in1=sumg[:, :NG], op0=ALU.mult, op1=ALU.mult)
                varg = epi.tile([P, EPI * G], f32, tag="varg")
                nc.vector.scalar_tensor_tensor(
                    out=varg[:, :NG], in0=sqg[:, :NG], scalar=1.0 / D,
                    in1=tmp[:, :NG], op0=ALU.mult, op1=ALU.subtract)
                stdg = epi.tile([P, EPI * G], f32, tag="stdg")
                nc.scalar.activation(out=stdg[:, :NG], in_=varg[:, :NG],
                                     func=ACT.Sqrt, bias=epsb[:, 0:1],
                                     scale=1.0)
                rstd = epi.tile([P, EPI * G], f32, tag="rstd")
                nc.vector.reciprocal(out=rstd[:, :NG], in_=stdg[:, :NG])
                mr = epi.tile([P, EPI * G], f32, tag="mr")
                nc.vector.scalar_tensor_tensor(
                    out=mr[:, :NG], in0=sumg[:, :NG], scalar=1.0 / D,
                    in1=rstd[:, :NG], op0=ALU.mult, op1=ALU.mult)
                xn = epi.tile([P, EPI * U], f32, tag="xn")
                xn3 = xn[:, :W].rearrange("p (g d) -> p g d", g=NG)
                rstdb = rstd[:, :NG].unsqueeze(2).broadcast_to([P, NG, D])
                mrb = mr[:, :NG].unsqueeze(2).broadcast_to([P, NG, D])
                nc.vector.tensor_mul(out=xn3, in0=fu3, in1=rstdb)
                nc.vector.tensor_sub(out=xn3, in0=xn3, in1=mrb)
                nc.vector.tensor_mul(out=xn[:, :W], in0=xn[:, :W],
                                     in1=gammab[:, :W])
                nc.vector.tensor_add(out=xn[:, :W], in0=xn[:, :W],
                                     in1=betab[:, :W])
                outf = epi.tile([P, EPI * U], f32, tag="outf")
                nc.scalar.activation(out=outf[:, :W], in_=xn[:, :W],
                                     func=ACT.Relu)
                rows_all = min(g_n * P, NPC - t0 * P)
                full_g = rows_all // P
                if full_g:
                    nc.sync.dma_start(
                        out=bass.AP(tensor=d_out, offset=t0 * P * U,
                                    ap=[[U, P], [P * U, full_g], [1, U]]),
                        in_=outf[:, :full_g * U].rearrange(
                            "p (g u) -> p g u", u=U))
                rem = rows_all - full_g * P
                if rem > 0:
                    nc.sync.dma_start(
                        out=bass.AP(tensor=d_out,
                                    offset=(t0 + full_g) * P * U,
                                    ap=[[U, rem], [1, U]]),
                        in_=outf[:rem, full_g * U:(full_g + 1) * U])

            # ------------- fusion + scatter emitter (deferred) -------------
            fstate = {"acc8": None, "fu32": None}

            def emit_fusion(t, msg, ohc):
                g_i = t % EPI
                g_n = min(EPI, NTS - (t - g_i))
                o_i = t % 8
                if o_i == 0:
                    fstate["acc8"] = pacc.tile([P, 512], f32, space="PSUM",
                                               tag="acc", name="acc8")
                acc8 = fstate["acc8"]
                acc = acc8[:, o_i * U:(o_i + 1) * U]
                for r0 in range(0, NSEG, 8):          # rounds of 8 segments
                    rn = min(8, NSEG - r0)
                    f6 = pscr.tile([P, 512], f32, space="PSUM", tag="scr")
                    for k in range(rn):
                        s = r0 + k
                        jj = s >> 1
                        h = s & 1
                        fo = slice(k * U, (k + 1) * U)
                        nc.tensor.matmul(
                            out=f6[:, fo],
                            lhsT=msg[:, h * HALF + jj * P:
                                     h * HALF + (jj + 1) * P],
                            rhs=fwa[:], start=True, stop=False)
                        nc.tensor.matmul(
                            out=f6[:, fo],
                            lhsT=msg[h * U:(h + 1) * U,
                                     2 * HALF + jj * P:2 * HALF + (jj + 1) * P],
                            rhs=fwb[h * U:(h + 1) * U, :], start=False,
                            stop=True)
                    fsb = fsp.tile([P, 512], f16, tag="fsb")
                    nc.scalar.activation(out=fsb[:, :rn * U],
                                         in_=f6[:, :rn * U], func=ACT.Copy)
                    for k in range(rn):
                        s = r0 + k
                        nc.tensor.matmul(
                            out=acc, lhsT=ohc[:, s * P:(s + 1) * P],
                            rhs=fsb[:, k * U:(k + 1) * U],
                            start=(s == 0), stop=(s == NSEG - 1))

                # acc evacuation (adds fusion bias)
                if g_i == 0:
                    fstate["fu32"] = epi.tile([P, EPI * U], f32, tag="fu32",
                                              name="fu32")
                fu32 = fstate["fu32"]
                if o_i == 7 or t == NTS - 1:
                    na = o_i + 1
                    nc.vector.tensor_add(
                        out=fu32[:].rearrange("p (g u) -> p g u", u=U)
                            [:, g_i - o_i:g_i + 1, :],
                        in0=acc8[:].rearrange("p (g u) -> p g u", u=U)
                            [:, 0:na, :],
                        in1=fbb[:].unsqueeze(1).broadcast_to([P, na, U]))
                if g_i == g_n - 1:
                    pending.append((t - g_i, g_n, fu32))

            # ---------------- main loop ----------------
            fusq = []
            for t in range(NTS):
                g_i = t % EPI            # index within epilogue batch

                if t == 0:
                    ef2c, srcc, ohc = pre0
                else:
                    ef2c, srcc = tile_dmas(t)
                    ohc = oh_dma(t)

                msg = msgp.tile([P, 3 * HALF], f16, tag="msg")
                off = 0
                for ci, sz in enumerate(cfg.chunks):
                    cs = slice(off, off + sz)
                    hp = pscr.tile([P, 512], f32, space="PSUM", tag="scr")
                    nc.tensor.matmul(out=hp[:, :sz], lhsT=w1bd[:],
                                     rhs=ef2c[:, cs], start=True, stop=True)
                    h2 = h2p.tile([P, 512], f16, tag="h2")
                    nc.scalar.activation(out=h2[:, :sz], in_=hp[:, :sz],
                                         func=ACT.Relu, bias=b1d[:, 0:1],
                                         scale=1.0)

                    # three channel segments: a1 (a-ch, first edges),
                    # a2 (a-ch, second edges), b (b-ch, both)
                    segs = (
                        (w2a[0:U, :], h2[0:U, :sz],
                         wka[0:F, :], srcc[0:F, cs], b2a, 0),
                        (w2a[U:2 * U, :], h2[U:2 * U, :sz],
                         wka[F:2 * F, :], srcc[F:2 * F, cs], b2a, 1),
                        (w2bd[:], h2[:, :sz], wkbd[:], srcc[:, cs], b2bd, 2),
                    )
                    for (wew, hrhs, wml, mrhs, b2v, k) in segs:
                        ewp_t = pew.tile([P, 512], f32, space="PSUM", tag="ew")
                        nc.tensor.matmul(out=ewp_t[:, :sz], lhsT=wew,
                                         rhs=hrhs, start=True, stop=True)
                        mlp_t = pml.tile([P, 512], f32, space="PSUM", tag="ml")
                        nc.tensor.matmul(out=mlp_t[:, :sz], lhsT=wml,
                                         rhs=mrhs, start=True, stop=True)
                        ews = ewsp.tile([P, 512], f16, tag="ews")
                        if t % 2 == 1 and (k == 2 or (k == 1 and ci == 0)):
                            # balance: odd tiles evacuate the b-segment on DVE
                            nc.vector.tensor_tensor(
                                out=ews[:, :sz], in0=ewp_t[:, :sz],
                                in1=b2v[:, 0:1].broadcast_to([P, sz]),
                                op=ALU.add)
                        else:
                            nc.scalar.activation(
                                out=ews[:, :sz], in_=ewp_t[:, :sz],
                                func=ACT.Identity, bias=b2v[:, 0:1], scale=1.0)
                        # msg = (ew+b2) * ml  (ml read straight from PSUM)
                        nc.vector.tensor_mul(
                            out=msg[:, k * HALF + off:k * HALF + off + sz],
                            in0=ews[:, :sz], in1=mlp_t[:, :sz])
                    off += sz
                    if ci == 0 and pending and g_i == 0:
                        emit_epi(*pending.pop(0))
                # deferred fusion rides after this tile's chunks so each
                # engine FIFO always has ready work queued ahead of it
                if fusq:
                    emit_fusion(*fusq.pop(0))
                fusq.append((t, msg, ohc))
            while fusq:
                emit_fusion(*fusq.pop(0))
            while pending:
                emit_epi(*pending.pop(0))

    nc.compile()
    return nc


# ---------------------------------------------------------------- execution

_CACHE = {}
_LAST_PERM = None


def unpermute(out_raw):
    return out_raw[_LAST_PERM] if _LAST_PERM is not None else out_raw


def _get_nc(cfg):
    key = (cfg.N, cfg.E, cfg.ncores, cfg.NSEG)
    if key not in _CACHE:
        _CACHE[key] = build_nc(cfg)
    return _CACHE[key]


def run(cfg, inputs, trace=False):
    from concourse.bass_utils import run_bass_kernel_spmd
    while True:
        try:
            in_maps = prepare_inputs(cfg, inputs)
            break
        except OverflowError as e:
            cfg = Cfg(cfg.N, cfg.E, cfg.ncores, int(e.args[0]))
    nc = _get_nc(cfg)
    res = run_bass_kernel_spmd(nc, in_maps, list(range(cfg.ncores)),
                               trace=trace)
    out = np.concatenate([r["out"] for r in res.results], axis=0)
    return unpermute(out.astype(np.float32)), res


def kernel(**inputs):
    out, _ = run(FULL, inputs)
    return out
